# revision 20
# baseline (speedup 1.0000x reference)
"""nn_AutoregressiveDecoder Trainium2 Bass kernel.

8-core SPMD decomposition of the 6-layer block-causal transformer decoder:
  - attention is head-parallel (1 of 8 heads per core), scores computed in
    [key, query] orientation so the PV matmul contracts over keys on the
    partition dim; softmax denominators ride along as an augmented ones
    column of V; block-causal structure skips never-allowed (key-tile, q)
    regions and fixes up diagonal tiles with per-partition 0/1 mask vectors.
  - an AllToAll mixes heads back so each core holds all 8 heads for its
    256-token slice; Wo projection, residuals, LayerNorms and the FFN are
    token-parallel; an AllGather rebuilds the replicated feature-major
    activation for the next layer's QKV.

All matmuls run in bf16 (fp32 PSUM accumulation); the residual stream and
LayerNorm math stay fp32.
"""

import math
import sys
import types

import ml_dtypes
import numpy as np

BF16 = ml_dtypes.bfloat16

DIM = 256
HEADS = 8
DH = DIM // HEADS
LAYERS = 6
NUM_AGENT = 33
NUM_POLYGON = 64
E = NUM_AGENT + NUM_POLYGON  # 97
MAX_T = 101
B = 1
T = 20
FFN = 4 * DIM
EPS = 1e-5

S = T * E  # 1940
SP = 2048  # padded sequence
NB = SP // 256  # 8 token blocks of 256
NKT = SP // 128  # 16 key tiles of 128
NC = 8  # cores

TSTEP = np.array([min(t // E, T - 1) if t < S else T for t in range(SP)])  # pad -> T


def _tstep(tok):
    return tok // E if tok < S else T  # pad tokens get an impossible timestep


# per key-tile: first allowed query column (start of first timestep present in tile)
def _qc0(kt):
    min_tk = (kt * 128) // E
    return E * min_tk


# mask segments: (kt, col_start, col_end) plus the 0/1 row-vector, for regions where
# some rows of key-tile kt are disallowed (or padding) for queries in [col_start,col_end)
def _segments():
    segs = []
    for kt in range(NKT):
        rows = np.arange(kt * 128, kt * 128 + 128)
        row_t = np.array([_tstep(r) for r in rows])
        t_lo = _qc0(kt) // E
        for t in range(t_lo, T):
            c0 = E * t
            c1 = E * (t + 1) if t < T - 1 else SP
            m = ((row_t <= t)).astype(np.float32)
            if m.min() > 0.5:
                continue  # all rows allowed, no mask needed
            segs.append((kt, c0, c1, m))
    return segs


SEGS = _segments()


def _host_prepare(inputs):
    """Build per-core input maps (numpy) from the full problem inputs."""
    x = np.asarray(inputs["x"], np.float32)
    spatial = np.asarray(inputs["spatial_emb"], np.float32)
    temporal = np.asarray(inputs["temporal_emb"], np.float32)
    xe = x[0] + spatial[None, :, :] + temporal[:T, None, :]
    xe = xe.reshape(S, DIM)
    xp = np.zeros((SP, DIM), np.float32)
    xp[:S] = xe

    # feature-major blocks: x0[b*256 + f, j] = xp[b*256 + j, f]
    x0 = np.ascontiguousarray(
        xp.reshape(NB, 256, DIM).transpose(0, 2, 1).reshape(SP, DIM)
    ).astype(BF16)

    Wq = np.asarray(inputs["Wq"], np.float32)
    Wk = np.asarray(inputs["Wk"], np.float32)
    Wv = np.asarray(inputs["Wv"], np.float32)
    Wo = np.asarray(inputs["Wo"], np.float32)
    bq = np.asarray(inputs["bq"], np.float32)
    bk = np.asarray(inputs["bk"], np.float32)
    bv = np.asarray(inputs["bv"], np.float32)
    bo = np.asarray(inputs["bo"], np.float32)
    W1 = np.asarray(inputs["W1"], np.float32)
    b1 = np.asarray(inputs["b1"], np.float32)
    W2 = np.asarray(inputs["W2"], np.float32)
    b2 = np.asarray(inputs["b2"], np.float32)
    g1 = np.asarray(inputs["ln1_g"], np.float32)
    be1 = np.asarray(inputs["ln1_b"], np.float32)
    g2 = np.asarray(inputs["ln2_g"], np.float32)
    be2 = np.asarray(inputs["ln2_b"], np.float32)

    scale = 1.0 / math.sqrt(DH)

    # replicated tensors
    def pack_kp(w, kt_count, mcols):  # [L, kt_count*128, mcols] -> [128, L*kt_count*mcols]
        out = np.empty((128, LAYERS * kt_count * mcols), np.float32)
        for l in range(LAYERS):
            for k in range(kt_count):
                out[:, (l * kt_count + k) * mcols : (l * kt_count + k + 1) * mcols] = w[
                    l, k * 128 : (k + 1) * 128, :
                ]
        return out

    wo_p = pack_kp(Wo, 2, 256).astype(BF16)
    w1_p = pack_kp(W1, 2, 1024).astype(BF16)
    w2_p = pack_kp(W2, 8, 256).astype(BF16)

    bob = np.empty((128, LAYERS * 256), np.float32)
    b2b = np.empty((128, LAYERS * 256), np.float32)
    gb = np.empty((128, LAYERS * 2 * 256), np.float32)
    bb = np.empty((128, LAYERS * 2 * 256), np.float32)
    b1v = np.empty((128, LAYERS * 8), np.float32)
    for l in range(LAYERS):
        bob[:, l * 256 : (l + 1) * 256] = bo[l][None, :]
        b2b[:, l * 256 : (l + 1) * 256] = b2[l][None, :]
        gb[:, (l * 2 + 0) * 256 : (l * 2 + 1) * 256] = g1[l][None, :]
        gb[:, (l * 2 + 1) * 256 : (l * 2 + 2) * 256] = g2[l][None, :]
        bb[:, (l * 2 + 0) * 256 : (l * 2 + 1) * 256] = be1[l][None, :]
        bb[:, (l * 2 + 1) * 256 : (l * 2 + 2) * 256] = be2[l][None, :]
        for h in range(8):
            b1v[:, l * 8 + h] = b1[l, h * 128 : (h + 1) * 128]
    bob = bob.astype(BF16)
    b2b = b2b.astype(BF16)
    gb = gb.astype(BF16)
    bb = bb.astype(BF16)

    masks = np.stack([m for (_, _, _, m) in SEGS], axis=1) if SEGS else np.zeros((128, 1), np.float32)

    sel = np.zeros((8, 256), np.float32)
    for h in range(8):
        sel[h, h * 32 : (h + 1) * 32] = 1.0

    eps1 = np.full((128, 1), EPS, np.float32)
    ident = np.eye(128, dtype=np.float32)

    in_maps = []
    for c in range(NC):
        h0 = c * DH  # this core's head feature offset
        wq_p = np.empty((128, LAYERS * 2 * DH), np.float32)
        wk_p = np.empty((128, LAYERS * 2 * DH), np.float32)
        wv_p = np.empty((128, LAYERS * 2 * DH), np.float32)
        qkbias = np.zeros((64, LAYERS), np.float32)
        bvb = np.empty((128, LAYERS * 512), np.float32)
        for l in range(LAYERS):
            for k in range(2):
                sl = slice((l * 2 + k) * DH, (l * 2 + k + 1) * DH)
                wq_p[:, sl] = Wq[l, k * 128 : (k + 1) * 128, h0 : h0 + DH] * scale
                wk_p[:, sl] = Wk[l, k * 128 : (k + 1) * 128, h0 : h0 + DH]
                wv_p[:, sl] = Wv[l, k * 128 : (k + 1) * 128, h0 : h0 + DH]
            qkbias[0:32, l] = bq[l, h0 : h0 + DH] * scale
            qkbias[32:64, l] = bk[l, h0 : h0 + DH]
            bvb[:, l * 512 : (l + 1) * 512] = np.tile(bv[l, h0 : h0 + DH], 16)[None, :]
        m = {
            "x0": x0,
            "xres0": np.ascontiguousarray(xp[c * 256 : (c + 1) * 256]),
            "wq": wq_p.astype(BF16),
            "wk": wk_p.astype(BF16),
            "wv": wv_p.astype(BF16),
            "wo": wo_p,
            "w1": w1_p,
            "w2": w2_p,
            "qkbias": qkbias,
            "bvb": bvb.astype(BF16),
            "bob": bob,
            "b2b": b2b,
            "gb": gb,
            "bb": bb,
            "b1v": b1v,
            "masks": masks,
            "sel": sel.astype(BF16),
            "eps1": eps1,
            "ident": ident,
        }
        in_maps.append(m)
    return in_maps


def _build_nc(debug=False):
    import concourse.bass as bass
    import concourse.mybir as mybir
    import concourse.tile as tile

    dt = mybir.dt
    F32 = dt.float32
    BF = dt.bfloat16
    AF = mybir.ActivationFunctionType
    OP = mybir.AluOpType

    nc = bass.Bass(num_devices=NC)
    rg = [list(range(NC))]

    # ---- dram parameters ----
    def par(name, shape, d=BF):
        return nc.declare_dram_parameter(name, shape, d, isOutput=False)

    x0_d = par("x0", [SP, DIM])
    xres0_d = par("xres0", [256, DIM], F32)
    wq_d = par("wq", [128, LAYERS * 2 * DH])
    wk_d = par("wk", [128, LAYERS * 2 * DH])
    wv_d = par("wv", [128, LAYERS * 2 * DH])
    wo_d = par("wo", [128, LAYERS * 2 * 256])
    w1_d = par("w1", [128, LAYERS * 2 * 1024])
    w2_d = par("w2", [128, LAYERS * 8 * 256])
    qkbias_d = par("qkbias", [64, LAYERS], F32)
    bvb_d = par("bvb", [128, LAYERS * 512])
    bob_d = par("bob", [128, LAYERS * 256])
    b2b_d = par("b2b", [128, LAYERS * 256])
    gb_d = par("gb", [128, LAYERS * 2 * 256])
    bb_d = par("bb", [128, LAYERS * 2 * 256])
    b1v_d = par("b1v", [128, LAYERS * 8], F32)
    nseg = max(1, len(SEGS))
    masks_d = par("masks", [128, nseg], F32)
    sel_d = par("sel", [8, 256])
    eps1_d = par("eps1", [128, 1], F32)
    ident_d = par("ident", [128, 128], F32)
    out_d = nc.declare_dram_parameter("out", [256, DIM], F32, isOutput=True)
    if debug:
        dbg_q = nc.declare_dram_parameter("dbg_q", [32, SP], BF, isOutput=True)
        dbg_k = nc.declare_dram_parameter("dbg_k", [32, SP], BF, isOutput=True)
        dbg_v = nc.declare_dram_parameter("dbg_v", [128, 16 * 33], BF, isOutput=True)
        dbg_a = nc.declare_dram_parameter("dbg_a", [33, SP], BF, isOutput=True)
        dbg_afm = nc.declare_dram_parameter("dbg_afm", [128, 2 * 256], BF, isOutput=True)
        dbg_amy = nc.declare_dram_parameter("dbg_amy", [128, 2 * 256], BF, isOutput=True)
        dbg_dn = nc.declare_dram_parameter("dbg_dn", [8, 256], BF, isOutput=True)
        dbg_a2a = nc.declare_dram_parameter("dbg_a2a", [8, 33, 256], BF, isOutput=True)
        dbg_y1 = nc.declare_dram_parameter("dbg_y1", [128, 2 * 256], F32, isOutput=True)
        dbg_y2 = nc.declare_dram_parameter("dbg_y2", [128, 2 * 256], F32, isOutput=True)

    # collective bounce buffers (per layer)
    a2a_in = [nc.dram_tensor(f"a2a_in{l}", [8, 33, 256], BF) for l in range(LAYERS)]
    a2a_out = [nc.dram_tensor(f"a2a_out{l}", [8, 33, 256], BF) for l in range(LAYERS)]
    ag_in = [nc.dram_tensor(f"ag_in{l}", [256, 256], BF) for l in range(LAYERS - 1)]
    ag_out = [
        nc.dram_tensor(f"ag_out{l}", [SP, 256], BF, addr_space="Shared")
        for l in range(LAYERS - 1)
    ]

    with tile.TileContext(nc) as tc:
        import contextlib

        ctx = contextlib.ExitStack()
        with ctx:
            pw = ctx.enter_context(tc.tile_pool(name="weights", bufs=1))
            px = ctx.enter_context(tc.tile_pool(name="acts", bufs=1))
            pqk = ctx.enter_context(tc.tile_pool(name="qk", bufs=1))
            pes = ctx.enter_context(tc.tile_pool(name="es", bufs=3))
            pxr = ctx.enter_context(tc.tile_pool(name="xres", bufs=3))
            pt = ctx.enter_context(tc.tile_pool(name="lnt", bufs=4))
            pst = ctx.enter_context(tc.tile_pool(name="stats", bufs=12))
            pmy = ctx.enter_context(tc.tile_pool(name="my", bufs=2))
            # PSUM budget is 8 banks (16KB/partition), statically pooled:
            # pool "A": 3 shared 4KB slots (6 banks) for all small/medium tiles;
            # pool "O": one 4KB slot (2 banks) for the PV accumulator per q-group.
            ps_a = ctx.enter_context(tc.tile_pool(name="ps_a", bufs=3, space="PSUM"))
            ps_o = ctx.enter_context(tc.tile_pool(name="ps_o", bufs=1, space="PSUM"))

            dma = nc.sync.dma_start

            # ---- persistent sbuf tensors ----
            def load(pool, dram, shape, d, rearr=None):
                t = pool.tile(shape, d, tag=f"w_{dram.name}")
                src = dram[:]
                if rearr is not None:
                    src = src.rearrange(*rearr[0], **rearr[1])
                dma(t[:], src)
                return t

            wq = load(pw, wq_d, [128, LAYERS * 2 * DH], BF)
            wk = load(pw, wk_d, [128, LAYERS * 2 * DH], BF)
            wv = load(pw, wv_d, [128, LAYERS * 2 * DH], BF)
            wo = load(pw, wo_d, [128, LAYERS * 2 * 256], BF)
            w1 = load(pw, w1_d, [128, LAYERS * 2 * 1024], BF)
            w2 = load(pw, w2_d, [128, LAYERS * 8 * 256], BF)
            qkbias = load(pw, qkbias_d, [64, LAYERS], F32)
            bvb = load(pw, bvb_d, [128, LAYERS * 512], BF)
            bob = load(pw, bob_d, [128, LAYERS * 256], BF)
            b2b = load(pw, b2b_d, [128, LAYERS * 256], BF)
            gb = load(pw, gb_d, [128, LAYERS * 2 * 256], BF)
            bb = load(pw, bb_d, [128, LAYERS * 2 * 256], BF)
            b1v = load(pw, b1v_d, [128, LAYERS * 8], F32)
            masks = load(pw, masks_d, [128, nseg], F32)
            sel = load(pw, sel_d, [8, 256], BF)
            eps1 = load(pw, eps1_d, [128, 1], F32)
            ident = load(pw, ident_d, [128, 128], F32)

            # activation tensors
            xfm = px.tile([128, NB * 2 * 256], BF)  # replicated feature-major x
            dma(xfm[:].rearrange("p (b k j) -> p b k j", b=NB, k=2), x0_d[:].rearrange("(b k p) j -> p b k j", b=NB, k=2, p=128))
            q_sb = pqk.tile([32, SP], BF)
            k_sb = pqk.tile([32, SP], BF)
            v_sb = pqk.tile([128, 16 * 33], BF)
            nc.gpsimd.memset(v_sb[:].rearrange("p (s n) -> p s n", s=16)[:, :, 32:33], 1.0)
            a_sb = pqk.tile([33, SP], BF)
            dn = pqk.tile([8, 256], BF)
            rec = pqk.tile([8, 256], F32)
            rec_bf = pqk.tile([8, 256], BF)
            a_my = pqk.tile([128, 2, 256], BF)
            afm = pqk.tile([128, 2, 256], BF)
            h_sb = pqk.tile([128, 8, 256], BF)
            xfm_my = pqk.tile([128, 2, 256], BF)
            yT = pqk.tile([128, 2, 256], BF)

            xres = pxr.tile([128, 2, 256], F32)
            dma(xres[:], xres0_d[:].rearrange("(st p) j -> p st j", p=128))

            def layer_norm(l, which, ps_in, xres_in, bias_bc, y_out, stride=256):
                """y = LN(ps_in[:, st*stride:+256] + bias_bc + xres_in), token-major."""
                s1 = pst.tile([128, 2], F32, tag="s1")
                s2 = pst.tile([128, 2], F32, tag="s2")
                ts = []
                for st in range(2):
                    t0 = pt.tile([128, 256], F32, tag="lt")
                    nc.vector.scalar_tensor_tensor(
                        t0[:], ps_in[:, st * stride : st * stride + 256], 0.0,
                        bias_bc[:, l * 256 : (l + 1) * 256],
                        op0=OP.add, op1=OP.add,
                    )
                    t1 = pt.tile([128, 256], F32, tag="lt2")
                    nc.vector.scalar_tensor_tensor(
                        t1[:], t0[:], 0.0, xres_in[:, st, :],
                        op0=OP.add, op1=OP.add, accum_out=s1[:, st : st + 1],
                    )
                    sq = pt.tile([128, 256], F32, tag="sq")
                    nc.scalar.activation(sq[:], t1[:], AF.Square, accum_out=s2[:, st : st + 1])
                    ts.append(t1)
                mu = pst.tile([128, 2], F32, tag="mu")
                nc.vector.tensor_scalar_mul(mu[:], s1[:], 1.0 / 256.0)
                musq = pst.tile([128, 2], F32, tag="musq")
                nc.vector.tensor_mul(musq[:], mu[:], mu[:])
                var = pst.tile([128, 2], F32, tag="var")
                nc.vector.scalar_tensor_tensor(
                    var[:], s2[:], 1.0 / 256.0, musq[:], op0=OP.mult, op1=OP.subtract
                )
                lnv = pst.tile([128, 2], F32, tag="lnv")
                nc.scalar.activation(lnv[:], var[:], AF.Ln, bias=eps1[:])
                rstd = pst.tile([128, 2], F32, tag="rstd")
                nc.scalar.activation(rstd[:], lnv[:], AF.Exp, scale=-0.5)
                gsl = gb[:, (l * 2 + which) * 256 : (l * 2 + which + 1) * 256]
                bsl = bb[:, (l * 2 + which) * 256 : (l * 2 + which + 1) * 256]
                for st in range(2):
                    t1 = ts[st]
                    nc.vector.tensor_scalar(
                        t1[:], t1[:], mu[:, st : st + 1], rstd[:, st : st + 1],
                        op0=OP.subtract, op1=OP.mult,
                    )
                    nc.vector.tensor_mul(y_out[:, st, :], t1[:], gsl)
                    nc.vector.tensor_add(y_out[:, st, :], y_out[:, st, :], bsl)

            for l in range(LAYERS):
                # ---- V projection (token-major, all 16 position blocks) ----
                psv = ps_a.tile([128, 512], F32, tag="psa")
                for pos in range(16):
                    b, h2 = pos // 2, pos % 2
                    for kt in range(2):
                        nc.tensor.matmul(
                            psv[:, pos * 32 : pos * 32 + 32],
                            lhsT=xfm[:, (b * 2 + kt) * 256 + h2 * 128 : (b * 2 + kt) * 256 + h2 * 128 + 128],
                            rhs=wv[:, (l * 2 + kt) * DH : (l * 2 + kt + 1) * DH],
                            start=(kt == 0), stop=(kt == 1),
                        )
                nc.vector.tensor_add(
                    v_sb[:].rearrange("p (s n) -> p s n", s=16)[:, :, 0:32],
                    psv[:].rearrange("p (s n) -> p s n", s=16),
                    bvb[:, l * 512 : (l + 1) * 512].rearrange("p (s n) -> p s n", s=16),
                )

                # ---- Q, K projections (feature-major, 1024-col halves) ----
                for (dst, w_sb, brow) in ((q_sb, wq, 0), (k_sb, wk, 32)):
                    for half in range(2):
                        psq = ps_a.tile([32, 1024], F32, tag="psa")
                        for b in range(half * 4, half * 4 + 4):
                            for kt in range(2):
                                nc.tensor.matmul(
                                    psq[:, (b - half * 4) * 256 : (b - half * 4 + 1) * 256],
                                    lhsT=w_sb[:, (l * 2 + kt) * DH : (l * 2 + kt + 1) * DH],
                                    rhs=xfm[:, (b * 2 + kt) * 256 : (b * 2 + kt + 1) * 256],
                                    start=(kt == 0), stop=(kt == 1),
                                )
                        nc.scalar.activation(
                            dst[:, half * 1024 : (half + 1) * 1024], psq[:],
                            AF.Identity, bias=qkbias[brow : brow + 32, l : l + 1],
                        )

                # ---- attention: per 1024-col query group: scores -> exp -> mask -> PV ----
                for qg in range(2):
                    ga, gb_ = qg * 1024, (qg + 1) * 1024
                    kts = [kt for kt in range(NKT) if _qc0(kt) < gb_]
                    pso = ps_o.tile([33, 1024], F32, tag="pso")
                    for kt in kts:
                        ha = max(_qc0(kt), ga)
                        pss = ps_a.tile([128, 1024], F32, tag="psa")
                        off = 0
                        while ha + off < gb_:
                            cw = min(512, gb_ - ha - off)
                            nc.tensor.matmul(
                                pss[:, off : off + cw],
                                lhsT=k_sb[:, kt * 128 : (kt + 1) * 128],
                                rhs=q_sb[:, ha + off : ha + off + cw],
                                start=True, stop=True,
                            )
                            off += cw
                        es = pes.tile([128, 1024], BF, tag="es")
                        nc.scalar.activation(es[:, 0 : gb_ - ha], pss[:, 0 : gb_ - ha], AF.Exp)
                        # mask fixups intersecting this group
                        for si, (skt, sa, sb_, _m) in enumerate(SEGS):
                            if skt != kt:
                                continue
                            ia, ib = max(sa, ha), min(sb_, gb_)
                            if ia >= ib:
                                continue
                            nc.vector.tensor_scalar_mul(
                                es[:, ia - ha : ib - ha], es[:, ia - ha : ib - ha],
                                masks[:, si : si + 1],
                            )
                        # PV accumulate. start=True clears has_written for the WHOLE
                        # bank, so kt==0 must cover each 512-col bank in one matmul;
                        # later kts accumulate arbitrary sub-regions.
                        if kt == 0:
                            for bank in range(2):
                                nc.tensor.matmul(
                                    pso[:, bank * 512 : (bank + 1) * 512],
                                    lhsT=v_sb[:, kt * 33 : kt * 33 + 33],
                                    rhs=es[:, bank * 512 : (bank + 1) * 512],
                                    start=True, stop=(kt == kts[-1]),
                                    skip_group_check=True,
                                )
                        else:
                            for qb in range(ha // 256, gb_ // 256):
                                ia, ib = max(qb * 256, ha), min(qb * 256 + 256, gb_)
                                if ia < ib:
                                    nc.tensor.matmul(
                                        pso[:, ia - ga : ib - ga],
                                        lhsT=v_sb[:, kt * 33 : kt * 33 + 33],
                                        rhs=es[:, ia - ha : ib - ha],
                                        start=False, stop=(kt == kts[-1]),
                                        skip_group_check=True,
                                    )
                    # unnormalized numerators + denominators -> sbuf
                    nc.scalar.activation(a_sb[:, ga:gb_], pso[:], AF.Identity)
                dma(
                    a2a_in[l][:].rearrange("q m j -> m q j"),
                    a_sb[:].rearrange("m (q j) -> m q j", q=8),
                )
                nc.gpsimd.collective_compute(
                    "AllToAll", OP.bypass, replica_groups=rg,
                    ins=[a2a_in[l][:]], outs=[a2a_out[l][:]],
                )
                for h in range(8):
                    g, hh = h // 4, h % 4
                    dma(
                        a_my[hh * 32 : (hh + 1) * 32, g, :],
                        a2a_out[l][h, 0:32, :],
                    )
                dma(dn[:], a2a_out[l][:, 32, :])

                # normalize: afm[kt] = a_my[kt] * bcast(1/denominator)
                nc.vector.reciprocal(rec[:], dn[:])
                nc.vector.tensor_copy(rec_bf[:], rec[:])
                for kt in range(2):
                    psr = ps_a.tile([128, 256], F32, tag="psa")
                    nc.tensor.matmul(
                        psr[:], lhsT=sel[:, kt * 128 : (kt + 1) * 128], rhs=rec_bf[:],
                        start=True, stop=True,
                    )
                    nc.vector.tensor_mul(afm[:, kt, :], a_my[:, kt, :], psr[:])

                # ---- Wo projection on my token slice + LN1 ----
                psw = ps_a.tile([128, 512], F32, tag="psa")
                for st in range(2):
                    for kt in range(2):
                        nc.tensor.matmul(
                            psw[:, st * 256 : (st + 1) * 256],
                            lhsT=afm[:, kt, st * 128 : st * 128 + 128],
                            rhs=wo[:, (l * 2 + kt) * 256 : (l * 2 + kt + 1) * 256],
                            start=(kt == 0), stop=(kt == 1),
                        )
                y1 = pxr.tile([128, 2, 256], F32, tag="xres")
                layer_norm(l, 0, psw, xres, bob, y1)

                # ---- FFN on my token slice ----
                for st in range(2):
                    for kt in range(2):
                        pst_t = ps_a.tile([128, 128], F32, tag="psa")
                        nc.tensor.transpose(
                            pst_t[:], y1[:, st, kt * 128 : kt * 128 + 128], ident[:]
                        )
                        nc.vector.tensor_copy(xfm_my[:, kt, st * 128 : st * 128 + 128], pst_t[:])
                # st groups in separate banks: start=True clears a whole bank, so
                # interleaved accumulation groups must not share one.
                psw2 = ps_a.tile([128, 1024], F32, tag="psa")
                for ht in range(8):
                    ph = ps_a.tile([128, 256], F32, tag="psa")
                    for kt in range(2):
                        nc.tensor.matmul(
                            ph[:],
                            lhsT=w1[:, (l * 2 + kt) * 1024 + ht * 128 : (l * 2 + kt) * 1024 + ht * 128 + 128],
                            rhs=xfm_my[:, kt, :],
                            start=(kt == 0), stop=(kt == 1),
                        )
                    nc.vector.tensor_scalar(
                        h_sb[:, ht, :], ph[:], b1v[:, l * 8 + ht : l * 8 + ht + 1], 0.0,
                        op0=OP.add, op1=OP.max,
                    )
                    for st in range(2):
                        nc.tensor.matmul(
                            psw2[:, st * 512 : st * 512 + 256],
                            lhsT=h_sb[:, ht, st * 128 : st * 128 + 128],
                            rhs=w2[:, (l * 8 + ht) * 256 : (l * 8 + ht + 1) * 256],
                            start=(ht == 0), stop=(ht == 7),
                        )
                y2 = pxr.tile([128, 2, 256], F32, tag="xres")
                layer_norm(l, 1, psw2, y1, b2b, y2, stride=512)

                if debug and l == 0:
                    dma(dbg_q[:], q_sb[:])
                    dma(dbg_k[:], k_sb[:])
                    dma(dbg_v[:], v_sb[:])
                    dma(dbg_a[:], a_sb[:])
                    dma(dbg_afm[:].rearrange("p (k j) -> p k j", k=2), afm[:])
                    dma(dbg_amy[:].rearrange("p (k j) -> p k j", k=2), a_my[:])
                    dma(dbg_dn[:], dn[:])
                    dma(dbg_a2a[:], a2a_out[l][:])
                    dma(dbg_y1[:].rearrange("p (st j) -> p st j", st=2), y1[:])
                    dma(dbg_y2[:].rearrange("p (st j) -> p st j", st=2), y2[:])

                if l < LAYERS - 1:
                    # transpose y2 slice -> feature-major, AllGather, rebuild xfm
                    for st in range(2):
                        for kt in range(2):
                            pst_t = ps_a.tile([128, 128], F32, tag="psa")
                            nc.tensor.transpose(
                                pst_t[:], y2[:, st, kt * 128 : kt * 128 + 128], ident[:]
                            )
                            nc.vector.tensor_copy(yT[:, kt, st * 128 : st * 128 + 128], pst_t[:])
                    dma(ag_in[l][:].rearrange("(k p) j -> p k j", p=128), yT[:])
                    nc.gpsimd.collective_compute(
                        "AllGather", OP.bypass, replica_groups=rg,
                        ins=[ag_in[l][:]], outs=[ag_out[l][:]],
                    )
                    dma(
                        xfm[:].rearrange("p (b k j) -> p b k j", b=NB, k=2),
                        ag_out[l][:].rearrange("(b k p) j -> p b k j", b=NB, k=2, p=128),
                    )
                    xres = y2
                else:
                    dma(out_d[:].rearrange("(st p) j -> p st j", p=128), y2[:])

    return nc


_NTFF_INSTALLED = False


def _install_ntff_hook():
    global _NTFF_INSTALLED
    if _NTFF_INSTALLED or "antenv.axon_hooks" in sys.modules:
        _NTFF_INSTALLED = True
        return
    mod = types.ModuleType("antenv.axon_hooks")
    state = {"hook": None}
    mod.set_axon_ntff_profile_hook = lambda h: state.__setitem__("hook", h)
    mod.get_axon_ntff_profile_hook = lambda: state["hook"]
    sys.modules["antenv.axon_hooks"] = mod
    try:
        from trn_agent_boot.trn_boot import _ntff_profile_via_ctypes

        mod.set_axon_ntff_profile_hook(
            _ntff_profile_via_ctypes("/opt/axon/libaxon_pjrt.so")
        )
    except Exception:
        pass
    _NTFF_INSTALLED = True


def _split_sync_waits(nc, maxw=1):
    """This walrus build rejects instructions carrying more than ~1 sem wait;
    spill excess waits onto preceding same-engine NoOps."""
    import concourse.mybir as mybir

    n_new = 0
    for fn in nc.m.functions:
        for blk in fn.blocks:
            out = []
            for inst in blk.instructions:
                si = inst.sync_info
                waits = list(si.on_wait) if (si and si.on_wait) else []
                if len(waits) > maxw:
                    for k in range(0, len(waits) - maxw, maxw):
                        nop = mybir.InstNoOp(name=f"WSPLIT-{n_new}", ins=[], outs=[])
                        nop.engine = inst.engine
                        nop.sync_info = mybir.SyncInfo(
                            on_wait=waits[k : k + maxw], on_update=[]
                        )
                        out.append(nop)
                        n_new += 1
                    si.on_wait = waits[len(waits) - maxw :]
                out.append(inst)
            blk.instructions = out
    return n_new


_RUN_STATE = {}


def run_on_device(inputs, trace=False):
    """Compile (cached per process) and run; returns (per-core results, exec_time_ns)."""
    _install_ntff_hook()
    from concourse.bass_utils import run_bass_kernel_spmd

    if "nc" not in _RUN_STATE:
        nc = _build_nc()
        _split_sync_waits(nc)
        _RUN_STATE["nc"] = nc
    nc = _RUN_STATE["nc"]
    in_maps = _host_prepare(inputs)
    res = run_bass_kernel_spmd(nc, in_maps, list(range(NC)), trace=trace)
    return res.results, res.exec_time_ns


def kernel(**inputs):
    results, _ = run_on_device(inputs, trace=False)
    full = np.concatenate([results[c]["out"] for c in range(NC)], axis=0)
    return np.ascontiguousarray(full[:S].reshape(B, T, E, DIM)).astype(np.float32)
